# revision 7
# baseline (speedup 1.0000x reference)
"""Trainium2 Bass kernel for the AutoCorrelation module (Autoformer-style).

Shapes (hardcoded): B=8, N=128, L=192, H=8, E=64, D=64.

Math: for each (b, n):
  corr-mean  c[tau] = sum_s <Q_{(s+tau)%L}, K_s>  over the flattened (h,e) dim
             = circular-diagonal sums of the Gram matrix G[s,u] = <K_s, Q_u>
  top-5 delays per node from batch-averaged c (host), softmax weights (host),
  output o[tau, hd]  = sum_j w_j * v[(tau+d_j)%L, hd]
                     = (A @ V)[tau, hd]  with the sparse shift-matrix A (host-built)

Device work (8 cores, node axis sharded, 16 nodes/core, all 8 batches local):
  kernel 1: per-(b,n) Gram matrices, fp16 in / fp16 out (fp32 PSUM accumulate).
            Verified on the actual data: fp16 keeps the top-5 selection exact
            on all 128 nodes (min perturbed margin 1.3e-4); bf16 flips nodes.
  kernel 2: per-(b,n) V^T-stationary shift-matrix matmul, fp16 in/out
Host work: transposes, diag-sums, top-k, softmax, A-matrix build, reassembly.

Everything is DMA-bound; streams are spread so each DGE ring carries ~25MB:
  kernel 1: K in (sync HWDGE), Q in (scalar HWDGE), G out (gpsimd SWDGE)
  kernel 2: V in (sync), O out (scalar, 6KB/partition contiguous runs),
            A in (gpsimd)
"""

import numpy as np

import concourse.bass as bass  # noqa: F401
import concourse.mybir as mybir
import concourse.tile as tile
from concourse import bacc

B, N, L, H, E, D = 8, 128, 192, 8, 64, 64
HE = H * E            # 512
HD = H * D            # 512
NCORES = 8
NLOC = N // NCORES    # 16 nodes per core
BN = B * NLOC         # 128 (b, n) pairs per core
TOPK = 5              # int(log(192))

F32 = mybir.dt.float32
F16 = mybir.dt.float16


def _build_corr_nc(bn_count=BN, num_devices=NCORES, group=16):
    """Per (b,n): G[s,u] = sum_d k[s,d]*q[u,d], fp16 in, fp32 PSUM, fp16 out.

    Inputs kx/qx[bn, p, c*192 + l] fp16 (d = c*128 + p packs the he dim;
    l is time) -> 1.5KB contiguous runs per (bn, p). One 3.1MB input DMA
    per group of 16 bn per ring (K on sync, Q on scalar) so the ~2us
    per-op completion bubble amortizes.
    Output g4[bn//16, s, b16, u] fp16 (6KB runs) on the gpsimd ring.
    """
    nc = bacc.Bacc(
        "TRN2",
        target_bir_lowering=False,
        debug=False,
        enable_asserts=False,
        num_devices=num_devices,
    )
    kx = nc.dram_tensor("kx", [bn_count, 128, 4 * L], F16, kind="ExternalInput").ap()
    qx = nc.dram_tensor("qx", [bn_count, 128, 4 * L], F16, kind="ExternalInput").ap()
    g4 = nc.dram_tensor(
        "g4", [bn_count // group, L, group, L], F16, kind="ExternalOutput"
    ).ap()

    assert bn_count % group == 0
    with tile.TileContext(nc) as tc:
        with (
            tc.tile_pool(name="kin", bufs=2) as kpool,
            tc.tile_pool(name="qin", bufs=2) as qpool,
            tc.tile_pool(name="g0out", bufs=2) as g0pool,
            tc.tile_pool(name="g1out", bufs=2) as g1pool,
            tc.tile_pool(name="ps", bufs=8, space="PSUM") as pspool,
        ):
            for gi in range(0, bn_count, group):
                qd = gi // group
                ktile = kpool.tile([128, group, 4 * L], F16)
                nc.sync.dma_start(
                    out=ktile[:],
                    in_=kx[gi : gi + group].rearrange("b p x -> p b x"),
                )
                qtile = qpool.tile([128, group, 4 * L], F16)
                nc.scalar.dma_start(
                    out=qtile[:],
                    in_=qx[gi : gi + group].rearrange("b p x -> p b x"),
                )

                gt0 = g0pool.tile([128, group, L], F16)
                gt1 = g1pool.tile([64, group, L], F16)
                for i in range(group):
                    ps = pspool.tile([128, 2 * L], F32)
                    # m-chunks: G rows [0:128] -> ps[:, 0:L]; [128:192] -> ps[0:64, L:]
                    for msl, osl in (
                        (slice(0, 128), slice(0, L)),
                        (slice(128, 192), slice(L, 2 * L)),
                    ):
                        mlen = msl.stop - msl.start
                        for c in range(4):
                            x0 = c * L
                            nc.tensor.matmul(
                                ps[0:mlen, osl],
                                lhsT=ktile[:, i, x0 + msl.start : x0 + msl.stop],
                                rhs=qtile[:, i, x0 : x0 + L],
                                start=(c == 0),
                                stop=(c == 3),
                            )
                    nc.vector.tensor_copy(gt0[:, i, :], ps[0:128, 0:L])
                    nc.scalar.copy(gt1[:, i, :], ps[0:64, L : 2 * L])

                # both HWDGE rings carry the input streams; outputs go SWDGE
                nc.gpsimd.dma_start(out=g4[qd, 0:128], in_=gt0[:])
                nc.gpsimd.dma_start(out=g4[qd, 128:192], in_=gt1[:])

    nc.compile()
    return nc


def _build_agg_nc(bn_count=BN, num_devices=NCORES):
    """Per (b,n): o[hd, tau] = sum_t' v[t', hd] * at[t', tau], fp16 in/out.

    V is the stationary operand (full 128-row hd-chunks), AT the moving
    one; output is hd-major, transposed back on the host. PSUM per bn =
    2 one-bank tiles, so 4 bn stay in flight.
    """
    nc = bacc.Bacc(
        "TRN2",
        target_bir_lowering=False,
        debug=False,
        enable_asserts=False,
        num_devices=num_devices,
    )
    oct_ = 8
    assert bn_count % oct_ == 0
    nit = bn_count // oct_
    # at8[it, kc, p, b8, t]: t' = kc*96 + p -> 3KB contiguous runs; kc is
    # OUTER in the SBUF tile so descriptors stay whole (HBM run == SBUF run)
    at8 = nc.dram_tensor(
        "at8", [nit, 2, 96, oct_, L], F16, kind="ExternalInput"
    ).ap()
    # v8x[it, kc, p, b8, d] -> 8KB runs
    v8x = nc.dram_tensor(
        "v8x", [nit, 2, 96, oct_, HD], F16, kind="ExternalInput"
    ).ap()
    # o8[it, p, b8, half, cc*L + l]: element (bn, hd=(half*2+cc)*128+p, tau=l)
    # -> 12KB contiguous per (it, p), matching the SBUF tile exactly
    o8 = nc.dram_tensor(
        "o8", [nit, 128, oct_, 2, 2 * L], F16, kind="ExternalOutput"
    ).ap()

    with tile.TileContext(nc) as tc:
        with (
            tc.tile_pool(name="ain", bufs=3) as apool,
            tc.tile_pool(name="vin", bufs=3) as vpool,
            tc.tile_pool(name="oout", bufs=3) as opool,
            tc.tile_pool(name="ps", bufs=8, space="PSUM") as pspool,
        ):
            for it in range(nit):
                atile = apool.tile([96, 2, oct_, L], F16)
                nc.gpsimd.dma_start(
                    out=atile[:],
                    in_=at8[it].rearrange("kc p b t -> p kc b t"),
                )
                vtile = vpool.tile([96, 2, oct_, HD], F16)
                nc.sync.dma_start(
                    out=vtile[:],
                    in_=v8x[it].rearrange("kc p b d -> p kc b d"),
                )

                # otile free layout: (b8, half, cc*L + l); hd-chunk c = half*2+cc
                otile = opool.tile([128, oct_, 2, 2 * L], F16)
                for i in range(oct_):
                    pss = [
                        pspool.tile([128, 2 * L], F32, name="ps", tag="ps")
                        for _ in range(2)
                    ]
                    for c in range(4):
                        ps = pss[c // 2][0:128, (c % 2) * L : (c % 2 + 1) * L]
                        for kc in range(2):
                            nc.tensor.matmul(
                                ps,
                                lhsT=vtile[:, kc, i, c * 128 : (c + 1) * 128],
                                rhs=atile[:, kc, i, :],
                                start=(kc == 0),
                                stop=(kc == 1),
                            )
                    nc.vector.tensor_copy(otile[:, i, 0, :], pss[0][:])
                    nc.scalar.copy(otile[:, i, 1, :], pss[1][:])

                # output on the scalar HWDGE ring (sync carries V, gpsimd A)
                nc.scalar.dma_start(out=o8[it], in_=otile[:])

    nc.compile()
    return nc


_NC_CACHE = {}


def _get_nc(name):
    if name not in _NC_CACHE:
        _NC_CACHE[name] = {"corr": _build_corr_nc, "agg": _build_agg_nc}[name]()
    return _NC_CACHE[name]


_JIT_CACHE = {}


def _run_spmd(nc, in_maps):
    """run_bass_kernel_spmd's axon path with the jitted executable cached
    per-module, so repeat kernel() calls don't re-trace/re-compile."""
    import jax
    import numpy as _np
    from jax.experimental.shard_map import shard_map
    from jax.sharding import Mesh, PartitionSpec

    from concourse import bass2jax

    key = id(nc)
    if key not in _JIT_CACHE:
        bass2jax.install_neuronx_cc_hook()
        partition_name = (
            nc.partition_id_tensor.name if nc.partition_id_tensor else None
        )
        in_names, out_names, out_avals = [], [], []
        for alloc in nc.m.functions[0].allocations:
            if not isinstance(alloc, mybir.MemoryLocationSet):
                continue
            name = alloc.memorylocations[0].name
            if alloc.kind == "ExternalInput":
                if name != partition_name:
                    in_names.append(name)
            elif alloc.kind == "ExternalOutput":
                out_names.append(name)
                out_avals.append(
                    jax.core.ShapedArray(
                        tuple(alloc.tensor_shape), mybir.dt.np(alloc.dtype)
                    )
                )
        n_params = len(in_names)
        all_in_names = in_names + out_names
        if partition_name is not None:
            all_in_names = all_in_names + [partition_name]

        def _body(*args):
            operands = list(args)
            if partition_name is not None:
                operands.append(bass2jax.partition_id_tensor())
            outs = bass2jax._bass_exec_p.bind(
                *operands,
                out_avals=tuple(out_avals),
                in_names=tuple(all_in_names),
                out_names=tuple(out_names),
                lowering_input_output_aliases=(),
                sim_require_finite=True,
                sim_require_nnan=True,
                nc=nc,
            )
            return tuple(outs)

        devices = jax.devices()[:NCORES]
        mesh = Mesh(_np.asarray(devices), ("core",))
        n_outs = len(out_names)
        sharded = jax.jit(
            shard_map(
                _body,
                mesh=mesh,
                in_specs=(PartitionSpec("core"),) * (n_params + n_outs),
                out_specs=(PartitionSpec("core"),) * n_outs,
                check_rep=False,
            ),
            donate_argnums=tuple(range(n_params, n_params + n_outs)),
            keep_unused=True,
        )
        _JIT_CACHE[key] = (sharded, in_names, out_names, out_avals)

    sharded, in_names, out_names, out_avals = _JIT_CACHE[key]
    concat_in = [
        np.concatenate([np.asarray(m[name]) for m in in_maps], axis=0)
        for name in in_names
    ]
    concat_zeros = [
        np.zeros((NCORES * a.shape[0], *a.shape[1:]), a.dtype) for a in out_avals
    ]
    out_arrs = sharded(*concat_in, *concat_zeros)
    return [
        {
            name: np.asarray(out_arrs[i]).reshape(NCORES, *out_avals[i].shape)[c]
            for i, name in enumerate(out_names)
        }
        for c in range(NCORES)
    ]


def _run_spmd_safe(nc, in_maps):
    try:
        return _run_spmd(nc, in_maps)
    except Exception:
        from concourse.bass_utils import run_bass_kernel_spmd

        return run_bass_kernel_spmd(
            nc, in_maps, core_ids=list(range(NCORES))
        ).results


# circular-diagonal gather index: DIAG_IDX[s, tau] = (s + tau) % L
_DIAG_IDX = (np.arange(L)[:, None] + np.arange(L)[None, :]) % L
_S_IDX = np.arange(L)[:, None]


def kernel(queries, keys, values, attn_mask=None, **_unused):
    queries = np.asarray(queries)
    keys = np.asarray(keys)
    values = np.asarray(values)

    # ---- host prep: per-core sharded, time-last transposed q/k, fp16 -------
    def _pack16(x):
        # [B,N,L,H,E] -> [B,N,128,4*L] fp16 with free dim x = c*L + l,
        # where the (h,e) dim d = c*128 + p
        xt = x.transpose(0, 1, 3, 4, 2).reshape(B, N, 4, 128, L)
        return (
            xt.transpose(0, 1, 3, 2, 4).reshape(B, N, 128, 4 * L).astype(np.float16)
        )

    ktx = _pack16(keys)
    qtx = _pack16(queries)

    in_maps1 = []
    for i in range(NCORES):
        sl = slice(i * NLOC, (i + 1) * NLOC)
        in_maps1.append(
            {
                "kx": np.ascontiguousarray(ktx[:, sl]).reshape(BN, 128, 4 * L),
                "qx": np.ascontiguousarray(qtx[:, sl]).reshape(BN, 128, 4 * L),
            }
        )

    nc1 = _get_nc("corr")
    res1 = _run_spmd_safe(nc1, in_maps1)

    # ---- host: diag sums -> mean_value, top-k, softmax ---------------------
    # g4[core, grp, s, b16, u] -> g_all[core, bn, s, u]
    g_all = np.stack([r["g4"] for r in res1])  # [NC, BN/16, L, 16, L] fp16
    g_all = g_all.transpose(0, 1, 3, 2, 4).reshape(NCORES, BN, L, L)
    c_all = g_all[:, :, _S_IDX, _DIAG_IDX].sum(axis=2, dtype=np.float64)  # [NC,BN,L]
    mean_value = (
        c_all.reshape(NCORES, B, NLOC, L).transpose(1, 0, 2, 3).reshape(B, N, L)
        / HE
    )
    z = mean_value.mean(axis=0)  # [N, L]
    # jax.lax.top_k semantics: descending, ties -> lowest index (stable)
    index = np.argsort(-z, axis=-1, kind="stable")[:, :TOPK]  # [N, K]
    w = np.take_along_axis(mean_value, index[None], axis=-1)  # [B, N, K]
    e = np.exp(w - w.max(axis=-1, keepdims=True))
    tmp_corr = e / e.sum(axis=-1, keepdims=True)  # [B, N, K]

    # ---- host: build A^T (shift matrices), shard v -------------------------
    # AT[b, n, t', tau] = w_j  where t' = (tau + d_j) % L
    pos = np.arange(L)
    rows = (pos[None, None, :] + index[:, :, None]) % L  # [N, K, L]
    AT = np.zeros((B, N, L, L), dtype=np.float32)
    bI = np.arange(B)[:, None, None, None]
    nI = np.arange(N)[None, :, None, None]
    AT[bI, nI, rows[None], pos[None, None, None, :]] = tmp_corr[:, :, :, None]
    AT = AT.astype(np.float16)

    v_flat = values.reshape(B, N, L, HD).astype(np.float16)

    in_maps2 = []
    for i in range(NCORES):
        sl = slice(i * NLOC, (i + 1) * NLOC)
        at_core = AT[:, sl].reshape(BN, L, L)
        # at8[it, kc, p, b8, t]: t' = kc*96+p
        at_core = at_core.reshape(BN // 8, 8, 2, 96, L).transpose(0, 2, 3, 1, 4)
        v_core = v_flat[:, sl].reshape(BN // 8, 8, 2, 96, HD).transpose(
            0, 2, 3, 1, 4
        )
        in_maps2.append(
            {
                "at8": np.ascontiguousarray(at_core),
                "v8x": np.ascontiguousarray(v_core),
            }
        )

    nc2 = _get_nc("agg")
    res2 = _run_spmd_safe(nc2, in_maps2)

    # o8[it, p, b8, half, cc, l] fp16: element (8*it+b8, hd=(half*2+cc)*128+p, tau=l)
    o_all = np.stack([r["o8"] for r in res2])  # [NC, BN/8, 128, 8, 2*2L]
    o_all = o_all.reshape(NCORES, BN // 8, 128, 8, 2, 2, L)
    o_all = (
        o_all.astype(np.float32)
        .transpose(0, 1, 3, 4, 5, 2, 6)  # [NC, quad, b4, half, cc, p, l]
        .reshape(NCORES, BN, HD, L)
    )
    out = (
        o_all.transpose(0, 1, 3, 2)  # [NC, BN, L, HD]
        .reshape(NCORES, B, NLOC, L, H, D)
        .transpose(1, 0, 2, 3, 4, 5)
        .reshape(B, N, L, H, D)
    )
    return np.ascontiguousarray(out.astype(np.float32))
